# revision 18
# baseline (speedup 1.0000x reference)
"""Trainium2 Bass kernel for DeepKernelRegressionModel.

Math (per core, X sharded by rows across 8 cores):
  Xf = MLP(X), Yf = MLP(Y)                        (3-layer relu MLP, H=32)
  K[i,m] = exp(-|Xf_i - Yf_m|^2 / 2)
         = exp(Xf_i . Yf_m - |Xf_i|^2/2 - |Yf_m|^2/2)
  out = (K @ Y_target) / (K @ 1)

All heavy matmuls run in bf16 (1 col/cycle on the PE vs 2 for f32r):
the host pre-transposes X/Y and pre-casts everything to bf16, the MLP
runs on bf16 features, and the kernel-matrix exponent is built from the
*rounded* features so the Gaussian kernel is self-consistent (the large
|Yf|^2 terms cancel against the dot product).  The Y-norm row is carried
in two bf16 rows (hi + lo) for fp32-class accuracy; the X-norm row is a
single bf16 row (its error is constant per output row and cancels in the
weight normalization).

Main loop per i-chunk of 512 X rows: ONE bf16 matmul per 128-row m-tile
produces the exponent (contraction 35 = 32 features + 2 y-norm rows +
1 x-norm row), ScalarE exp's it to bf16, and a second bf16 matmul
contracts with [Y_target, 1] over m.  The loop is software-pipelined
(mm1 of iter k+1 issues before mm2 of iter k) and m-tiles alternate PE
row groups 0/64 so LDWEIGHTS overlaps the previous matmul.  A burst of
dummy matmuls at kernel start warms the PE HAM clock gate while the
input DMAs land.
"""

import numpy as np
from contextlib import ExitStack

import ml_dtypes
import concourse.bass as bass
import concourse.tile as tile
from concourse import bacc, mybir

FP = mybir.dt.float32
FPR = mybir.dt.float32r
BF = mybir.dt.bfloat16
AF = mybir.ActivationFunctionType
BF_NP = ml_dtypes.bfloat16

D, H, T = 64, 32, 8
ZP = 16     # Y_target cols (8) + ones col + zero pad
N_CORES = 8
NROW = 35   # mm1 contraction rows: 32 feat + ynorm hi/lo + xnorm


def build_nc(n_sh, m_total, exp_group=3, **_ignored):
    """Build the Bass program for one core (SPMD: same program, all cores).

    n_sh: rows of X handled by this core. m_total: rows of Y (full).
    """
    assert n_sh % 512 == 0 and m_total % 2048 == 0
    MT = m_total // 128       # number of 128-row m-tiles
    NCH = m_total // 512      # number of 512-wide m-chunks (MLP)
    CCY = NCH // 4            # stacked col chunks (4 chunks share 128 parts)
    XG = n_sh // 4            # X stacked-chunk width
    IT = n_sh // 128          # i-tiles
    IC = n_sh // 512          # i-chunks
    ICW = 512

    def r(ap):
        return ap.bitcast(FPR)

    nc = bacc.Bacc("TRN2", target_bir_lowering=False, debug=False,
                   num_devices=N_CORES)

    XTd = nc.dram_tensor("XT", [D, n_sh], BF, kind="ExternalInput").ap()
    YTd = nc.dram_tensor("YT", [D, m_total], BF, kind="ExternalInput").ap()
    ZTd = nc.dram_tensor("ZT", [128, MT * ZP], BF, kind="ExternalInput").ap()
    W1d = nc.dram_tensor("W1", [D, H], BF, kind="ExternalInput").ap()
    W2d = nc.dram_tensor("W2", [128, H], BF, kind="ExternalInput").ap()
    W3d = nc.dram_tensor("W3", [128, H], BF, kind="ExternalInput").ap()
    Bd = nc.dram_tensor("Bs", [128, 3], FP, kind="ExternalInput").ap()
    Id = nc.dram_tensor("ident", [128, 128], BF, kind="ExternalInput").ap()
    Ifd = nc.dram_tensor("identf", [ZP, ZP], FP, kind="ExternalInput").ap()
    NHd = nc.dram_tensor("neghalf", [128, 2], FP, kind="ExternalInput").ap()
    ORd = nc.dram_tensor("onesrow", [2, m_total], BF, kind="ExternalInput").ap()
    OUTd = nc.dram_tensor("out", [n_sh, T], FP, kind="ExternalOutput").ap()

    with tile.TileContext(nc) as tc, ExitStack() as ctx:
        const = ctx.enter_context(tc.tile_pool(name="const", bufs=1))
        big = ctx.enter_context(tc.tile_pool(name="big", bufs=1))
        scr = ctx.enter_context(tc.tile_pool(name="scr", bufs=1))

        # ---- PE warm-up: dummy matmuls with no DMA dependencies keep the
        # HAM activity window busy while the input DMAs land, so the PE
        # clock is at 2.4 GHz when real work starts.
        wsrc = scr.tile([128, 512], BF, tag="wsrc")
        nc.vector.memset(wsrc[:], 0.0)

        def warm_burst(n, name):
            # back-to-back dummy matmuls at 100% PE duty: flips the HAM
            # clock gate (pipelined real work has small gaps and won't)
            with tc.tile_pool(name=name, bufs=1, space="PSUM") as wpool:
                wp = wpool.tile([128, 512], FP, name=name + "t")
                for k in range(n):
                    nc.tensor.matmul(wp[:], lhsT=wsrc[:, 0:128], rhs=wsrc[:],
                                     start=True, stop=True,
                                     skip_group_check=True,
                                     tile_position=(0, 0))

        warm_burst(18, "warm0")

        # ---- constants (issue order: unblock X MLP, then Y MLP, zt last)
        w1s = const.tile([D, H], BF)
        nc.sync.dma_start(w1s[:], W1d[:])
        bs = const.tile([128, 3], FP)
        nc.sync.dma_start(bs[:], Bd[:])
        xTs = big.tile([D, n_sh], BF)        # X^T (host pre-transposed)
        nc.sync.dma_start(xTs[:], XTd[:])
        w2s = const.tile([128, H], BF)
        nc.sync.dma_start(w2s[:], W2d[:])
        w3s = const.tile([128, H], BF)
        nc.sync.dma_start(w3s[:], W3d[:])
        yTs = big.tile([D, m_total], BF)     # Y^T
        qw = m_total // 4
        for q in range(4):
            nc.sync.dma_start(yTs[:, qw * q:qw * q + qw], YTd[:, qw * q:qw * q + qw])
        ident = const.tile([128, 128], BF)
        nc.sync.dma_start(ident[:], Id[:])
        identf = const.tile([ZP, ZP], FP)
        nc.sync.dma_start(identf[:], Ifd[:])
        nh = const.tile([128, 2], FP)
        nc.sync.dma_start(nh[:].bitcast(FPR), NHd[:].bitcast(FPR))
        zt = const.tile([128, MT * ZP], BF)
        nc.sync.dma_start(zt[:], ZTd[:])

        # persistent big tensors
        yft = big.tile([128, m_total], BF)   # rows 0-34 aug A, 64-98 aug B
        xft = big.tile([128, n_sh], BF)
        yfs = big.tile([128, 512 * CCY], BF)   # Y features, stacked
        sqy = big.tile([128, 512 * CCY], FP)   # squared features (exact)
        for rb in (0, 64):  # ones rows: no producer deps, land early
            nc.sync.dma_start(yft[rb + 34:rb + 35, :], ORd[0:1, :])
            nc.sync.dma_start(xft[rb + 32:rb + 34, 0:n_sh], ORd[0:2, 0:n_sh])

        with (
            tc.tile_pool(name="mlp_psum", bufs=2, space="PSUM") as mpp,
            tc.tile_pool(name="np_psum", bufs=1, space="PSUM") as npp,
            tc.tile_pool(name="acts", bufs=2) as actp,
        ):
            # ---------------- phase X: MLP (4 chunks, stacked) ----------------
            # runs first so xft is ready well before the main loop starts
            hx1 = mpp.tile([128, XG], FP, tag="hp", bufs=4)
            for g in range(4):
                nc.tensor.matmul(hx1[32 * g:32 * g + 32, :],
                                 tile_position=(0, 32 * g),
                                 lhsT=w1s[:],
                                 rhs=xTs[:, XG * g:XG * g + XG],
                                 start=True, stop=True,
                                 skip_group_check=True)
            hx1s = scr.tile([128, XG], BF, tag="hxs1")
            nc.scalar.activation(hx1s[:], hx1[:], AF.Relu, bias=bs[:, 0:1])
            hx2 = mpp.tile([128, XG], FP, tag="hp", bufs=4)
            for g in range(4):
                nc.tensor.matmul(hx2[32 * g:32 * g + 32, :],
                                 tile_position=(32 * g, 32 * g),
                                 lhsT=w2s[32 * g:32 * g + 32, :],
                                 rhs=hx1s[32 * g:32 * g + 32, :],
                                 start=True, stop=True,
                                 skip_group_check=True)
            hx2s = scr.tile([128, XG], BF, tag="hxs2")
            nc.scalar.activation(hx2s[:], hx2[:], AF.Relu, bias=bs[:, 1:2])
            hx3 = mpp.tile([128, XG], FP, tag="hp", bufs=4)
            for g in range(4):
                nc.tensor.matmul(hx3[32 * g:32 * g + 32, :],
                                 tile_position=(32 * g, 32 * g),
                                 lhsT=w3s[32 * g:32 * g + 32, :],
                                 rhs=hx2s[32 * g:32 * g + 32, :],
                                 start=True, stop=True,
                                 skip_group_check=True)
            xfs = scr.tile([128, XG], BF, tag="xfs")
            nc.scalar.activation(xfs[:], hx3[:], AF.Relu, bias=bs[:, 2:3])
            sqx = scr.tile([128, XG], FP, tag="sqx")
            nc.vector.tensor_mul(sqx[:].bitcast(FPR), xfs[:], xfs[:])
            for rb in (0, 64):  # write both PE row groups directly
                for g in range(4):
                    nc.gpsimd.dma_start(xft[rb:rb + 32, XG * g:XG * g + XG],
                                        xfs[32 * g:32 * g + 32, :])
            # X norms -> single bf16 row 34 (dup 2-col outputs: f32r matmuls
            # need even moving free size + even psum offsets)
            xnp = npp.tile([128, 2 * IT], FP, tag="ynp")
            for it in range(IT):
                g, s = it // (XG // 128), it % (XG // 128)
                col = 128 * s
                nc.tensor.matmul(xnp[:, 2 * it:2 * it + 2],
                                 tile_position=(32 * g, 0),
                                 lhsT=r(sqx[32 * g:32 * g + 32, col:col + 128]),
                                 rhs=r(nh[32 * g:32 * g + 32, :]),
                                 start=True, stop=True,
                                 skip_group_check=True)
            xnhi = scr.tile([128, 2 * IT], BF, tag="xnhi")
            nc.vector.tensor_copy(xnhi[:], xnp[:])
            xtr = npp.tile([2 * IT, 128], BF, tag="ytr")
            nc.tensor.matmul(xtr[:], lhsT=xnhi[:], rhs=ident[:],
                             is_transpose=True, start=True, stop=True,
                             skip_group_check=True)
            xtrs = scr.tile([2 * IT, 128], BF, tag="xtrs")
            nc.vector.tensor_copy(xtrs[:], xtr[:])
            xtrs_v = xtrs.rearrange("(t k) c -> t k c", k=2)
            for rb in (0, 64):
                nc.gpsimd.dma_start(
                    xft[rb + 34:rb + 35, :].rearrange("o (t c) -> o t c", c=128),
                    xtrs_v[:, 0:1, :])

            # ---------------- phase Y: MLP (stacked 4x) ----------------
            # chunk ch (512 m's) -> group cg = ch%4, col chunk cc = ch//4.
            # 4-chunk blocks (one cc each) pipeline mm/relu across blocks;
            # norm matmuls run in-block, and norm rows are transposed and
            # DMA'd per half so the main loop's first groups unblock early.
            yfs_v = yfs.rearrange("p (cc c) -> p cc c", c=512)
            NBLK = (NCH + 3) // 4
            half = (NBLK + 1) // 2
            for hb in range((NBLK + half - 1) // half):
                blks = range(half * hb, min(half * hb + half, NBLK))
                for blk in blks:
                    chs = list(range(4 * blk, min(4 * blk + 4, NCH)))
                    cc = chs[0] // 4
                    h1p = mpp.tile([128, 512], FP, tag="hp", bufs=4)
                    for ch in chs:
                        cg = ch % 4
                        nc.tensor.matmul(h1p[32 * cg:32 * cg + 32, :],
                                         lhsT=w1s[:],
                                         rhs=yTs[:, 512 * ch:512 * ch + 512],
                                         start=True, stop=True,
                                         skip_group_check=True,
                                         tile_position=(0, 32 * cg))
                    h1s = actp.tile([128, 512], BF, tag="hs")
                    nc.scalar.activation(h1s[:], h1p[:], AF.Relu, bias=bs[:, 0:1])
                    h2p = mpp.tile([128, 512], FP, tag="hp", bufs=4)
                    for ch in chs:
                        cg = ch % 4
                        nc.tensor.matmul(h2p[32 * cg:32 * cg + 32, :],
                                         tile_position=(32 * cg, 32 * cg),
                                         lhsT=w2s[32 * cg:32 * cg + 32, :],
                                         rhs=h1s[32 * cg:32 * cg + 32, :],
                                         start=True, stop=True,
                                         skip_group_check=True)
                    h2s = actp.tile([128, 512], BF, tag="hs")
                    nc.scalar.activation(h2s[:], h2p[:], AF.Relu, bias=bs[:, 1:2])
                    h3p = mpp.tile([128, 512], FP, tag="hp", bufs=4)
                    for ch in chs:
                        cg = ch % 4
                        nc.tensor.matmul(h3p[32 * cg:32 * cg + 32, :],
                                         tile_position=(32 * cg, 32 * cg),
                                         lhsT=w3s[32 * cg:32 * cg + 32, :],
                                         rhs=h2s[32 * cg:32 * cg + 32, :],
                                         start=True, stop=True,
                                         skip_group_check=True)
                    nc.scalar.activation(yfs[:, 512 * cc:512 * cc + 512],
                                         h3p[:], AF.Relu, bias=bs[:, 2:3])
                    nc.vector.tensor_mul(sqy[:, 512 * cc:512 * cc + 512].bitcast(FPR),
                                         yfs[:, 512 * cc:512 * cc + 512],
                                         yfs[:, 512 * cc:512 * cc + 512])
                    # feature rows for this block (both PE row groups)
                    for rb in (0, 64):
                        yft_v = yft[rb:rb + 32, :].rearrange(
                            "p (cc gc) -> p cc gc", gc=2048)
                        for cg in range(4):
                            nc.gpsimd.dma_start(
                                yft_v[:, cc:cc + 1, 512 * cg:512 * cg + 512],
                                yfs_v[32 * cg:32 * cg + 32, cc:cc + 1, :])
                # ---- norms for this half ----
                mts = [mt for blk in blks
                       for mt in range(16 * blk, min(16 * blk + 16, MT))]
                nmts = len(mts)
                ynp = npp.tile([128, 2 * nmts], FP, tag="ynp")
                for k, mt in enumerate(mts):
                    ch, s = mt // 4, mt % 4
                    cg, cc = ch % 4, ch // 4
                    col = 512 * cc + 128 * s
                    nc.tensor.matmul(ynp[:, 2 * k:2 * k + 2],
                                     tile_position=(32 * cg, 0),
                                     lhsT=r(sqy[32 * cg:32 * cg + 32, col:col + 128]),
                                     rhs=r(nh[32 * cg:32 * cg + 32, :]),
                                     start=True, stop=True,
                                     skip_group_check=True)
                ynhi = scr.tile([128, 2 * nmts], BF, tag=f"ynhi{hb}")
                ynlo = scr.tile([128, 2 * nmts], BF, tag=f"ynlo{hb}")
                nc.vector.tensor_copy(ynhi[:], ynp[:])
                nc.vector.tensor_sub(ynlo[:], ynp[:], ynhi[:])
                ytr = npp.tile([2 * nmts, 256], BF, tag="ytr")
                nc.tensor.matmul(ytr[:, 0:128], lhsT=ynhi[:], rhs=ident[:],
                                 is_transpose=True, start=True, stop=False,
                                 skip_group_check=True)
                nc.tensor.matmul(ytr[:, 128:256], lhsT=ynlo[:], rhs=ident[:],
                                 is_transpose=True, start=False, stop=True,
                                 skip_group_check=True)
                ytrs = scr.tile([2 * nmts, 256], BF, tag=f"ytrs{hb}")
                nc.vector.tensor_copy(ytrs[:], ytr[:])
                ytrs_v = ytrs.rearrange("(t k) c -> t k c", k=2)
                m0 = 128 * mts[0]
                mw = 128 * nmts
                for rb in (0, 64):
                    nc.gpsimd.dma_start(
                        yft[rb + 32:rb + 33, m0:m0 + mw].rearrange(
                            "o (t c) -> o t c", c=128),
                        ytrs_v[:, 0:1, 0:128])
                    nc.gpsimd.dma_start(
                        yft[rb + 33:rb + 34, m0:m0 + mw].rearrange(
                            "o (t c) -> o t c", c=128),
                        ytrs_v[:, 0:1, 128:256])
        # ---------------- main loop ----------------
        # re-warm the PE: the MLP phase's dependency gaps re-throttle HAM
        warm_burst(9, "warm1")
        groups = []
        mt = 0
        while mt < MT:
            groups.append(list(range(mt, min(mt + exp_group, MT))))
            mt += exp_group

        with (
            tc.tile_pool(name="gbuf", bufs=2, space="PSUM") as gpool,
            tc.tile_pool(name="accp", bufs=2, space="PSUM") as apool,
            tc.tile_pool(name="ebuf", bufs=3) as epool,
            tc.tile_pool(name="fin", bufs=2) as finp,
        ):
            # software-pipelined: issue mm1(iter k+1) before mm2(iter k) so
            # the PE streams the next exponent while ScalarE exp's this one
            iters = [(ic, gi) for ic in range(IC) for gi in range(len(groups))]
            accs = {}
            pend = None  # (ic, grp, eb)

            def do_mm1(ic, grp):
                if grp is groups[0]:
                    accs[ic] = apool.tile([128, ICW], FP, tag="acc",
                                          name=f"acc{ic}")
                gp = gpool.tile([128, 512 * exp_group], FP, tag="g")
                for t, mt in enumerate(grp):
                    rg = 64 * (mt % 2)
                    nc.tensor.matmul(
                        gp[:, 512 * t:512 * t + 512],
                        tile_position=(rg, 0),
                        lhsT=yft[rg:rg + NROW, 128 * mt:128 * mt + 128],
                        rhs=xft[rg:rg + NROW, ICW * ic:ICW * ic + ICW],
                        start=True, stop=True)
                eb = epool.tile([128, 512 * exp_group], BF, tag="e")
                w = 512 * len(grp)
                nc.scalar.activation(eb[:, :w], gp[:, :w], AF.Exp)
                return eb

            def do_mm2(ic, grp, eb):
                for t, mt in enumerate(grp):
                    nc.tensor.matmul(
                        accs[ic][0:ZP, :],
                        tile_position=(0, 0),
                        lhsT=zt[:, ZP * mt:ZP * mt + ZP],
                        rhs=eb[:, 512 * t:512 * t + 512],
                        start=(mt == 0), stop=(mt == MT - 1),
                        skip_group_check=True)

            def do_fold(ic):
                # fold 4 col-group accumulators via transpose-accumulate
                acc_s = finp.tile([ZP, ICW], FP, tag="accs")
                nc.vector.tensor_copy(acc_s[:], accs[ic][0:ZP, :])
                ot = apool.tile([128, 4 * ZP], FP, tag="acc", name=f"ot{ic}")
                for q in range(4):
                    nc.tensor.matmul(
                        ot[:, ZP * q:ZP * q + ZP],
                        tile_position=(0, 0),
                        lhsT=acc_s[0:ZP, 128 * q:128 * q + 128],
                        rhs=identf[:],
                        is_transpose=True,
                        start=(q == 0), stop=(q == 3),
                        skip_group_check=True)
                for q in range(4):
                    rec = finp.tile([128, 1], FP, tag="rec")
                    nc.vector.reciprocal(rec[:], ot[:, ZP * q + T:ZP * q + T + 1])
                    res = finp.tile([128, T], FP, tag="res")
                    nc.vector.tensor_scalar_mul(res[:], ot[:, ZP * q:ZP * q + T],
                                                rec[:])
                    nc.gpsimd.dma_start(
                        OUTd[ICW * ic + 128 * q:ICW * ic + 128 * q + 128, :],
                        res[:])

            for (ic, gi) in iters:
                eb = do_mm1(ic, groups[gi])
                if pend is not None:
                    do_mm2(*pend)
                    if pend[1] is groups[-1]:
                        do_fold(pend[0])
                pend = (ic, groups[gi], eb)
            do_mm2(*pend)
            do_fold(pend[0])
    nc.compile()
    return nc


def make_in_maps(X, Y, Y_target, W1, b1, W2, b2, W3, b3, n_cores=N_CORES):
    f32 = lambda a: np.ascontiguousarray(np.asarray(a, dtype=np.float32))
    bf = lambda a: np.ascontiguousarray(np.asarray(a, dtype=np.float32).astype(BF_NP))
    X, Y, Y_target = f32(X), f32(Y), f32(Y_target)
    b1, b2, b3 = f32(b1), f32(b2), f32(b3)
    m_total = Y.shape[0]
    n_sh = X.shape[0] // n_cores
    MT = m_total // 128
    Zm = np.zeros((m_total, ZP), np.float32)
    Zm[:, :T] = Y_target
    Zm[:, T] = 1.0
    # pre-tiled [128, MT*ZP]: ZT[p, mt*ZP + c] = Zm[128*mt + p, c]
    ZT = np.transpose(Zm.reshape(MT, 128, ZP), (1, 0, 2)).reshape(128, MT * ZP)
    Bs = np.stack([np.tile(b1, 4), np.tile(b2, 4), np.tile(b3, 4)], axis=1)
    common = dict(
        YT=bf(Y.T), ZT=bf(ZT), W1=bf(W1),
        W2=bf(np.tile(W2, (4, 1))), W3=bf(np.tile(W3, (4, 1))),
        Bs=np.ascontiguousarray(Bs),
        ident=bf(np.eye(128, dtype=np.float32)),
        identf=np.eye(ZP, dtype=np.float32),
        neghalf=np.full((128, 2), -0.5, np.float32),
        onesrow=np.ones((2, m_total), BF_NP),
    )
    return [dict(common, XT=bf(X[c * n_sh:(c + 1) * n_sh].T))
            for c in range(n_cores)]


_NC_CACHE = {}


def _get_nc(n_sh, m_total):
    key = (n_sh, m_total)
    if key not in _NC_CACHE:
        _NC_CACHE[key] = build_nc(n_sh, m_total)
    return _NC_CACHE[key]


def kernel(X, Y, Y_target, W1, b1, W2, b2, W3, b3):
    from concourse.bass_utils import run_bass_kernel_spmd

    in_maps = make_in_maps(X, Y, Y_target, W1, b1, W2, b2, W3, b3)
    n_sh = in_maps[0]["XT"].shape[1]
    nc = _get_nc(n_sh, np.asarray(Y).shape[0])
    res = run_bass_kernel_spmd(nc, in_maps, core_ids=list(range(N_CORES)))
    return np.concatenate([res.results[c]["out"] for c in range(N_CORES)], axis=0)
